# revision 1
# baseline (speedup 1.0000x reference)
"""Trainium2 Bass kernel for nn_NormalizedDistanceLoss.

Math: for x in R^{N x D}, with sq_i = ||x_i||^2, the strict-upper-triangle
sum of pairwise squared distances collapses algebraically:

    sum_{i<j} (sq_i + sq_j - 2 x_i.x_j) = N * S - ||s||^2

where S = sum_i sq_i and s = sum_i x_i (column sums).  So the loss

    loss = sum_masked_dist / (sqrt(max_i sq_i) * N(N-1)/2)

needs only one pass over x: per-row squared norms (for S and the max)
and column sums (for s).  Each of the 8 cores reduces its 1024-row block;
the host combines the tiny per-core partials (8x(128x8) rowsq, 8x512 colsum).

Per-core device kernel (block = 1024 x 512 f32, laid out as p t d with
p=128 partitions, t=8 row-tiles, d=512):
  - DMA the block into SBUF in chunks (pipelined).
  - ACT engine: Square activation with accum_out -> per-row squared norms.
  - DVE: strided reduce over t -> per-partition column sums (128 x 512).
  - PE: ones(128,1)^T @ cpart -> column sums (1 x 512) in PSUM.
"""

import sys

if "/opt/trn_rl_repo" not in sys.path:
    sys.path.insert(0, "/opt/trn_rl_repo")

import numpy as np

import concourse.bass as bass
import concourse.tile as tile
from concourse import bacc, mybir

N = 8192
D = 512
NCORES = 8
ROWS = N // NCORES  # 1024 rows per core
P = 128
T = ROWS // P  # 8 row-tiles of 128
DMA_CHUNKS = 4

_nc_cache = []


def _build_nc():
    nc = bacc.Bacc(
        "TRN2",
        target_bir_lowering=False,
        debug=False,
        num_devices=NCORES,
    )
    x_dram = nc.dram_tensor("x_blk", [ROWS, D], mybir.dt.float32, kind="ExternalInput")
    rowsq_dram = nc.dram_tensor(
        "rowsq", [P, T], mybir.dt.float32, kind="ExternalOutput"
    )
    colsum_dram = nc.dram_tensor(
        "colsum", [1, D], mybir.dt.float32, kind="ExternalOutput"
    )

    with tile.TileContext(nc) as tc:
        with (
            tc.tile_pool(name="xpool", bufs=1) as xpool,
            tc.tile_pool(name="scratch", bufs=2) as scratch,
            tc.tile_pool(name="stats", bufs=1) as stats,
            tc.tile_pool(name="psum", bufs=1, space=bass.MemorySpace.PSUM) as psum_pool,
        ):
            X = xpool.tile([P, T, D], mybir.dt.float32)
            x_r = x_dram[:].rearrange("(t p) d -> p t d", p=P)
            t_per = T // DMA_CHUNKS
            for c in range(DMA_CHUNKS):
                sl = slice(c * t_per, (c + 1) * t_per)
                nc.sync.dma_start(X[:, sl, :], x_r[:, sl, :])

            ones = stats.tile([P, 1], mybir.dt.float32)
            nc.vector.memset(ones[:], 1.0)

            # Per-row squared norms on ACT: out scratch is thrown away,
            # accum_out gives the row sums of squares.
            rowsq = stats.tile([P, T], mybir.dt.float32)
            for t in range(T):
                xsq = scratch.tile([P, D], mybir.dt.float32, tag="xsq")
                nc.scalar.activation(
                    xsq[:],
                    X[:, t, :],
                    mybir.ActivationFunctionType.Square,
                    accum_out=rowsq[:, t : t + 1],
                )

            # Column sums: reduce over t on DVE, then collapse partitions on PE.
            cpart = stats.tile([P, D], mybir.dt.float32)
            nc.vector.tensor_reduce(
                cpart[:],
                X[:].rearrange("p t d -> p d t"),
                axis=mybir.AxisListType.X,
                op=mybir.AluOpType.add,
            )
            ps = psum_pool.tile([1, D], mybir.dt.float32)
            nc.tensor.matmul(ps[:], ones[:], cpart[:], start=True, stop=True)
            colsum = stats.tile([1, D], mybir.dt.float32)
            nc.scalar.copy(colsum[:], ps[:])

            nc.sync.dma_start(rowsq_dram[:], rowsq[:])
            nc.sync.dma_start(colsum_dram[:], colsum[:])

    nc.compile()
    return nc


def get_nc():
    if not _nc_cache:
        _nc_cache.append(_build_nc())
    return _nc_cache[0]


def combine_partials(rowsq, colsum):
    """rowsq: (NCORES, P, T), colsum: (NCORES, 1, D) -> scalar loss."""
    S = rowsq.sum(dtype=np.float64)
    maxsq = float(rowsq.max())
    s = colsum.sum(axis=(0, 1), dtype=np.float64)
    count = N * (N - 1) // 2
    loss = (N * S - s @ s) / (np.sqrt(maxsq) * count)
    return np.float32(loss)


def kernel(x):
    from concourse.bass_utils import run_bass_kernel_spmd

    x = np.ascontiguousarray(np.asarray(x), dtype=np.float32)
    assert x.shape == (N, D), x.shape
    nc = get_nc()
    in_maps = [{"x_blk": x[c * ROWS : (c + 1) * ROWS]} for c in range(NCORES)]
    res = run_bass_kernel_spmd(nc, in_maps, list(range(NCORES)))
    rowsq = np.stack([r["rowsq"] for r in res.results])
    colsum = np.stack([r["colsum"] for r in res.results])
    return combine_partials(rowsq, colsum)


# revision 2
# speedup vs baseline: 1.0983x; 1.0983x over previous
"""Trainium2 Bass kernel for nn_NormalizedDistanceLoss.

Math: for x in R^{N x D}, with sq_i = ||x_i||^2, the strict-upper-triangle
sum of pairwise squared distances collapses algebraically:

    sum_{i<j} (sq_i + sq_j - 2 x_i.x_j) = N * S - ||s||^2

where S = sum_i sq_i and s = sum_i x_i (column sums).  So the loss

    loss = sum_masked_dist / (sqrt(max_i sq_i) * N(N-1)/2)

needs only one pass over x: per-row squared norms (for S and the max)
and column sums (for s).  Each of the 8 cores reduces its 1024-row block;
the host combines tiny per-core partials.

Per-core device kernel (block = 1024 x 512 f32):
  - SBUF layout (128, 8, 512): partition p holds DRAM rows p*8..p*8+7,
    i.e. 16KB contiguous per partition -> full-rate DMA.  4 chunked DMAs
    (2 row-tiles each) so compute pipelines with transfer.
  - Row squared norms via one fused square+row-sum op per 512-wide tile:
    ACT (activation Square + accum_out) for even tiles, DVE
    (scalar_tensor_tensor mult + accum_out) for odd tiles.
  - Column sums as two per-engine accumulators: DVE adds odd tiles,
    GpSimd adds even tiles (3 adds each).  No matmul / PSUM needed;
    the host collapses the 128 partitions.
"""

import sys

if "/opt/trn_rl_repo" not in sys.path:
    sys.path.insert(0, "/opt/trn_rl_repo")

import numpy as np

import concourse.bass as bass
import concourse.tile as tile
from concourse import bacc, mybir

N = 8192
D = 512
NCORES = 8
ROWS = N // NCORES  # 1024 rows per core
P = 128
T = ROWS // P  # 8 row-tiles of 512
NCHUNKS = 4
TPC = T // NCHUNKS  # row-tiles per DMA chunk

_nc_cache = []


def _build_nc():
    f32 = mybir.dt.float32
    nc = bacc.Bacc(
        "TRN2",
        target_bir_lowering=False,
        debug=False,
        num_devices=NCORES,
    )
    x_dram = nc.dram_tensor("x_blk", [ROWS, D], f32, kind="ExternalInput")
    rowsq_a_dram = nc.dram_tensor("rowsq_a", [P, T // 2], f32, kind="ExternalOutput")
    rowsq_b_dram = nc.dram_tensor("rowsq_b", [P, T // 2], f32, kind="ExternalOutput")
    accd_dram = nc.dram_tensor("acc_d", [P, D], f32, kind="ExternalOutput")
    accg_dram = nc.dram_tensor("acc_g", [P, D], f32, kind="ExternalOutput")

    with tile.TileContext(nc) as tc:
        with (
            tc.tile_pool(name="xpool", bufs=1) as xpool,
            tc.tile_pool(name="scr_a", bufs=2) as scr_a,
            tc.tile_pool(name="scr_b", bufs=2) as scr_b,
            tc.tile_pool(name="stats", bufs=1) as stats,
        ):
            X = xpool.tile([P, T, D], f32)
            # partition p <- DRAM rows p*T .. p*T+T-1 (contiguous 16KB)
            x_r = x_dram[:].rearrange("(p t) d -> p t d", p=P)

            rowsq_a = stats.tile([P, T // 2], f32)  # even tiles (ACT)
            rowsq_b = stats.tile([P, T // 2], f32)  # odd tiles (DVE)
            acc_d = stats.tile([P, D], f32)  # odd-tile colsum partial (DVE)
            acc_g = stats.tile([P, D], f32)  # even-tile colsum partial (GpSimd)

            for c in range(NCHUNKS):
                sl = slice(c * TPC, (c + 1) * TPC)
                nc.sync.dma_start(X[:, sl, :], x_r[:, sl, :])

                te, to = 2 * c, 2 * c + 1  # even (ACT), odd (DVE) row-tile
                xsq_a = scr_a.tile([P, D], f32, tag="xsq_a")
                nc.scalar.activation(
                    xsq_a[:],
                    X[:, te, :],
                    mybir.ActivationFunctionType.Square,
                    accum_out=rowsq_a[:, c : c + 1],
                )
                xsq_b = scr_b.tile([P, D], f32, tag="xsq_b")
                nc.vector.scalar_tensor_tensor(
                    out=xsq_b[:],
                    in0=X[:, to, :],
                    scalar=1.0,
                    in1=X[:, to, :],
                    op0=mybir.AluOpType.mult,
                    op1=mybir.AluOpType.mult,
                    accum_out=rowsq_b[:, c : c + 1],
                )
                # column-sum accumulators
                if c == 1:
                    nc.vector.tensor_add(acc_d[:], X[:, 1, :], X[:, 3, :])
                    nc.gpsimd.tensor_add(acc_g[:], X[:, 0, :], X[:, 2, :])
                elif c >= 2:
                    nc.vector.tensor_add(acc_d[:], acc_d[:], X[:, to, :])
                    nc.gpsimd.tensor_add(acc_g[:], acc_g[:], X[:, te, :])

            nc.sync.dma_start(rowsq_a_dram[:], rowsq_a[:])
            nc.sync.dma_start(accd_dram[:], acc_d[:])
            nc.scalar.dma_start(rowsq_b_dram[:], rowsq_b[:])
            nc.scalar.dma_start(accg_dram[:], acc_g[:])

    nc.compile()
    return nc


def get_nc():
    if not _nc_cache:
        _nc_cache.append(_build_nc())
    return _nc_cache[0]


def combine_partials(rowsq_parts, acc_parts):
    """rowsq_parts: iterable of (P, T//2) arrays; acc_parts: iterable of
    (P, D) colsum partials -> scalar loss.  Row order is irrelevant for
    sum/max, so no reindexing is needed."""
    S = 0.0
    maxsq = -np.inf
    for r in rowsq_parts:
        S += r.sum(dtype=np.float64)
        maxsq = max(maxsq, float(r.max()))
    s = np.zeros(D, dtype=np.float64)
    for a in acc_parts:
        s += a.sum(axis=0, dtype=np.float64)
    count = N * (N - 1) // 2
    loss = (N * S - s @ s) / (np.sqrt(maxsq) * count)
    return np.float32(loss)


def kernel(x):
    from concourse.bass_utils import run_bass_kernel_spmd

    x = np.ascontiguousarray(np.asarray(x), dtype=np.float32)
    assert x.shape == (N, D), x.shape
    nc = get_nc()
    in_maps = [{"x_blk": x[c * ROWS : (c + 1) * ROWS]} for c in range(NCORES)]
    res = run_bass_kernel_spmd(nc, in_maps, list(range(NCORES)))
    rowsq_parts = [r[k] for r in res.results for k in ("rowsq_a", "rowsq_b")]
    acc_parts = [r[k] for r in res.results for k in ("acc_d", "acc_g")]
    return combine_partials(rowsq_parts, acc_parts)


# revision 5
# speedup vs baseline: 1.1266x; 1.0258x over previous
"""Trainium2 Bass kernel for nn_NormalizedDistanceLoss.

Math: for x in R^{N x D}, with sq_i = ||x_i||^2, the strict-upper-triangle
sum of pairwise squared distances collapses algebraically:

    sum_{i<j} (sq_i + sq_j - 2 x_i.x_j) = N * S - ||s||^2

where S = sum_i sq_i and s = sum_i x_i (column sums).  So the loss

    loss = sum_masked_dist / (sqrt(max_i sq_i) * N(N-1)/2)

needs only one pass over x: per-row squared norms (for S and the max)
and column sums (for s).  Each of the 8 cores reduces its 1024-row block;
the host combines tiny per-core partials (a few KB per core).

Per-core device kernel (block = 1024 x 512 f32):
  - SBUF layout (128, 8, 512): partition p holds DRAM rows p*8..p*8+7
    (16KB contiguous per partition).  4 chunked DMAs (2 row-tiles each)
    split across BOTH HWDGE rings (sync + scalar) so transfers overlap.
  - Row squared norms: one fused square+row-sum op per 512-wide tile;
    ACT (Square activation + accum_out) for even tiles, DVE
    (scalar_tensor_tensor + accum_out) for odd tiles.
  - Column sums: DVE adds each tile pair into a bf16 pair tile; the
    otherwise-idle PE contracts the 128 partitions with a ones-vector
    matmul, accumulating all pairs in one PSUM bank.  bf16 pair rounding
    perturbs the final loss at ~1e-8 relative - far below fp32 noise.
"""

import sys

if "/opt/trn_rl_repo" not in sys.path:
    sys.path.insert(0, "/opt/trn_rl_repo")

import numpy as np

import concourse.bass as bass
import concourse.tile as tile
from concourse import bacc, mybir

N = 8192
D = 512
NCORES = 8
ROWS = N // NCORES  # 1024 rows per core
P = 128
T = ROWS // P  # 8 row-tiles of 512
NCHUNKS = 4
TPC = T // NCHUNKS  # row-tiles per DMA chunk (2)

_nc_cache = []


def _build_nc():
    f32 = mybir.dt.float32
    bf16 = mybir.dt.bfloat16
    nc = bacc.Bacc(
        "TRN2",
        target_bir_lowering=False,
        debug=False,
        num_devices=NCORES,
    )
    x_dram = nc.dram_tensor("x_blk", [ROWS, D], f32, kind="ExternalInput")
    rowsq_dram = nc.dram_tensor("rowsq", [P, T], f32, kind="ExternalOutput")
    colsum_dram = nc.dram_tensor("colsum", [1, D], f32, kind="ExternalOutput")

    with tile.TileContext(nc) as tc:
        with (
            tc.tile_pool(name="xpool", bufs=1) as xpool,
            tc.tile_pool(name="scr_a", bufs=2) as scr_a,
            tc.tile_pool(name="scr_b", bufs=2) as scr_b,
            tc.tile_pool(name="pairs", bufs=4) as pairs,
            tc.tile_pool(name="stats", bufs=1) as stats,
            tc.tile_pool(name="psum", bufs=1, space=bass.MemorySpace.PSUM) as psum_pool,
        ):
            X = xpool.tile([P, T, D], f32)
            # partition p <- DRAM rows p*T .. p*T+T-1 (contiguous 16KB)
            x_r = x_dram[:].rearrange("(p t) d -> p t d", p=P)

            # One chunk per DMA channel so transfers never queue behind a
            # completion semaphore: sync ring 3 tiles, scalar ring 3 tiles,
            # gpsimd SWDGE 2 tiles.
            nc.sync.dma_start(X[:, 0:3, :], x_r[:, 0:3, :])
            nc.scalar.dma_start(X[:, 3:6, :], x_r[:, 3:6, :])
            nc.gpsimd.dma_start(X[:, 6:8, :], x_r[:, 6:8, :])

            rowsq = stats.tile([P, T], f32)  # cols 0..3 ACT, 4..7 DVE
            ps = psum_pool.tile([1, D], f32)
            onesb = nc.const_aps.tensor(1.0, [P, 1], bf16)

            for c in range(NCHUNKS):
                te, to = 2 * c, 2 * c + 1
                pair = pairs.tile([P, D], bf16, tag="pair")
                nc.vector.tensor_add(pair[:], X[:, te, :], X[:, to, :])
                nc.tensor.matmul(
                    ps[:], onesb, pair[:], start=(c == 0), stop=(c == NCHUNKS - 1)
                )
                xsq_a = scr_a.tile([P, D], f32, tag="xsq_a")
                nc.scalar.activation(
                    xsq_a[:],
                    X[:, te, :],
                    mybir.ActivationFunctionType.Square,
                    accum_out=rowsq[:, c : c + 1],
                )
                xsq_b = scr_b.tile([P, D], f32, tag="xsq_b")
                nc.vector.scalar_tensor_tensor(
                    out=xsq_b[:],
                    in0=X[:, to, :],
                    scalar=1.0,
                    in1=X[:, to, :],
                    op0=mybir.AluOpType.mult,
                    op1=mybir.AluOpType.mult,
                    accum_out=rowsq[:, 4 + c : 5 + c],
                )

            colsum = stats.tile([1, D], f32)
            nc.scalar.copy(colsum[:], ps[:])

            nc.sync.dma_start(rowsq_dram[:], rowsq[:])
            nc.scalar.dma_start(colsum_dram[:], colsum[:])

    nc.compile()
    return nc


def get_nc():
    if not _nc_cache:
        _nc_cache.append(_build_nc())
    return _nc_cache[0]


def combine_partials(rowsq_parts, colsum_parts):
    """rowsq_parts: per-core (P, T//2) row-squared-norm arrays; colsum_parts:
    per-core (1, D) column sums -> scalar loss.  Row order is irrelevant
    for sum/max, so no reindexing is needed."""
    S = 0.0
    maxsq = -np.inf
    for r in rowsq_parts:
        S += r.sum(dtype=np.float64)
        maxsq = max(maxsq, float(r.max()))
    s = np.zeros(D, dtype=np.float64)
    for cs in colsum_parts:
        s += cs.reshape(-1).astype(np.float64)
    count = N * (N - 1) // 2
    loss = (N * S - s @ s) / (np.sqrt(maxsq) * count)
    return np.float32(loss)


def kernel(x):
    from concourse.bass_utils import run_bass_kernel_spmd

    x = np.ascontiguousarray(np.asarray(x), dtype=np.float32)
    assert x.shape == (N, D), x.shape
    nc = get_nc()
    in_maps = [{"x_blk": x[c * ROWS : (c + 1) * ROWS]} for c in range(NCORES)]
    res = run_bass_kernel_spmd(nc, in_maps, list(range(NCORES)))
    rowsq_parts = [r["rowsq"] for r in res.results]
    colsum_parts = [r["colsum"] for r in res.results]
    return combine_partials(rowsq_parts, colsum_parts)


# revision 6
# speedup vs baseline: 1.3328x; 1.1831x over previous
"""Trainium2 Bass kernel for nn_NormalizedDistanceLoss.

Math: for x in R^{N x D}, with sq_i = ||x_i||^2, the strict-upper-triangle
sum of pairwise squared distances collapses algebraically:

    sum_{i<j} (sq_i + sq_j - 2 x_i.x_j) = N * S - ||s||^2

where S = sum_i sq_i and s = sum_i x_i (column sums).  So the loss

    loss = sum_masked_dist / (sqrt(max_i sq_i) * N(N-1)/2)

needs only one pass over x: per-row squared norms (for S and the max)
and column sums (for s).  Each of the 8 cores reduces its 1024-row block;
the host combines tiny per-core partials (a few KB per core).

Per-core device kernel (block = 1024 x 512 f32):
  - SBUF layout (128, 8, 512): partition p holds DRAM rows p*8..p*8+7
    (16KB contiguous per partition).  4 chunked DMAs (2 row-tiles each)
    split across BOTH HWDGE rings (sync + scalar) so transfers overlap.
  - Row squared norms: one fused square+row-sum op per 512-wide tile;
    ACT (Square activation + accum_out) for even tiles, DVE
    (scalar_tensor_tensor + accum_out) for odd tiles.
  - Column sums: DVE adds each tile pair into a bf16 pair tile; the
    otherwise-idle PE contracts the 128 partitions with a ones-vector
    matmul, accumulating all pairs in one PSUM bank.  bf16 pair rounding
    perturbs the final loss at ~1e-8 relative - far below fp32 noise.
"""

import sys

if "/opt/trn_rl_repo" not in sys.path:
    sys.path.insert(0, "/opt/trn_rl_repo")

import numpy as np

import concourse.bass as bass
import concourse.tile as tile
from concourse import bacc, mybir

N = 8192
D = 512
NCORES = 8
ROWS = N // NCORES  # 1024 rows per core
P = 128
T = ROWS // P  # 8 row-tiles of 512
NCHUNKS = 4
TPC = T // NCHUNKS  # row-tiles per DMA chunk (2)

_nc_cache = []


def _build_nc():
    f32 = mybir.dt.float32
    bf16 = mybir.dt.bfloat16
    nc = bacc.Bacc(
        "TRN2",
        target_bir_lowering=False,
        debug=False,
        num_devices=NCORES,
    )
    x_dram = nc.dram_tensor("x_blk", [ROWS, D], f32, kind="ExternalInput")
    rowsq_dram = nc.dram_tensor("rowsq", [P, T], f32, kind="ExternalOutput")
    colsum_dram = nc.dram_tensor("colsum", [1, D], f32, kind="ExternalOutput")

    with tile.TileContext(nc) as tc:
        with (
            tc.tile_pool(name="xpool", bufs=1) as xpool,
            tc.tile_pool(name="scr_a", bufs=2) as scr_a,
            tc.tile_pool(name="scr_b", bufs=2) as scr_b,
            tc.tile_pool(name="pairs", bufs=4) as pairs,
            tc.tile_pool(name="stats", bufs=1) as stats,
            tc.tile_pool(name="psum", bufs=1, space=bass.MemorySpace.PSUM) as psum_pool,
        ):
            X = xpool.tile([P, T, D], f32)
            # partition p <- DRAM rows p*T .. p*T+T-1 (contiguous 16KB)
            x_r = x_dram[:].rearrange("(p t) d -> p t d", p=P)

            rowsq = stats.tile([P, T], f32)  # cols 0..3 ACT, 4..7 DVE
            ps = psum_pool.tile([1, D], f32)
            onesb = nc.const_aps.tensor(1.0, [P, 1], bf16)

            # 4 chunks of 2 row-tiles, alternating between the two HWDGE
            # rings (sync / scalar) so two transfers are always in flight:
            # wave 1 = chunks 0,1 (tiles 0-3), wave 2 = chunks 2,3.
            for c in range(NCHUNKS):
                sl = slice(c * TPC, (c + 1) * TPC)
                eng = nc.sync if c % 2 == 0 else nc.scalar
                eng.dma_start(X[:, sl, :], x_r[:, sl, :])

                te, to = 2 * c, 2 * c + 1
                pair = pairs.tile([P, D], bf16, tag="pair")
                nc.vector.tensor_add(pair[:], X[:, te, :], X[:, to, :])
                nc.tensor.matmul(
                    ps[:], onesb, pair[:], start=(c == 0), stop=(c == NCHUNKS - 1)
                )
                xsq_a = scr_a.tile([P, D], f32, tag="xsq_a")
                nc.scalar.activation(
                    xsq_a[:],
                    X[:, te, :],
                    mybir.ActivationFunctionType.Square,
                    accum_out=rowsq[:, c : c + 1],
                )
                xsq_b = scr_b.tile([P, D], f32, tag="xsq_b")
                nc.vector.scalar_tensor_tensor(
                    out=xsq_b[:],
                    in0=X[:, to, :],
                    scalar=1.0,
                    in1=X[:, to, :],
                    op0=mybir.AluOpType.mult,
                    op1=mybir.AluOpType.mult,
                    accum_out=rowsq[:, 4 + c : 5 + c],
                )

            colsum = stats.tile([1, D], f32)
            nc.scalar.copy(colsum[:], ps[:])

            nc.sync.dma_start(rowsq_dram[:], rowsq[:])
            nc.scalar.dma_start(colsum_dram[:], colsum[:])

    nc.compile()
    return nc


def get_nc():
    if not _nc_cache:
        _nc_cache.append(_build_nc())
    return _nc_cache[0]


def combine_partials(rowsq_parts, colsum_parts):
    """rowsq_parts: per-core (P, T//2) row-squared-norm arrays; colsum_parts:
    per-core (1, D) column sums -> scalar loss.  Row order is irrelevant
    for sum/max, so no reindexing is needed."""
    S = 0.0
    maxsq = -np.inf
    for r in rowsq_parts:
        S += r.sum(dtype=np.float64)
        maxsq = max(maxsq, float(r.max()))
    s = np.zeros(D, dtype=np.float64)
    for cs in colsum_parts:
        s += cs.reshape(-1).astype(np.float64)
    count = N * (N - 1) // 2
    loss = (N * S - s @ s) / (np.sqrt(maxsq) * count)
    return np.float32(loss)


def kernel(x):
    from concourse.bass_utils import run_bass_kernel_spmd

    x = np.ascontiguousarray(np.asarray(x), dtype=np.float32)
    assert x.shape == (N, D), x.shape
    nc = get_nc()
    in_maps = [{"x_blk": x[c * ROWS : (c + 1) * ROWS]} for c in range(NCORES)]
    res = run_bass_kernel_spmd(nc, in_maps, list(range(NCORES)))
    rowsq_parts = [r["rowsq"] for r in res.results]
    colsum_parts = [r["colsum"] for r in res.results]
    return combine_partials(rowsq_parts, colsum_parts)
